# revision 14
# baseline (speedup 1.0000x reference)
"""Trainium2 Bass kernel for nn_Half_Graph (GNN message passing block).

Data-parallel over batch: core b processes image b (B=8 across 8 cores).

Planar layout: SBUF partition 10*g + c <-> (channel c, pixel group g) with
G=6 groups of 6144 pixels; a 10-channel tensor occupies 60 partitions.
128-partition tiles hold two such 60-row halves at [0:60] and [64:124].

All convs are 1x1 -> matmuls with block-diagonal stationaries. Host builds
bf16 concat tensors [first; second] per conv block so each conv1 is a
SINGLE matmul pass (K=128). Partition row 60 of every concat tensor is a
constant 1.0, and stationary row 60 carries the folded BN / GRU biases, so
biases ride along in the matmul for free. The attention premultiply reads
a PE-broadcast copy of the attention maps (computed once for all chunks up
front). The inter-block message sum is a relu+add chain on Pool/DVE
reading conv2 PSUM tiles directly. GRU output uses out = h + u*(c - h).

Host side pre-transposes/concats/casts (cheap, not part of measured
device time) so every DMA is a plain 2D bf16 slice.
"""

import sys

for _p in ("/opt/trn_rl_repo", "/root/.axon_site/_ro/trn_rl_repo"):
    if _p not in sys.path:
        sys.path.insert(0, _p)

import numpy as np

import concourse.bass as bass
import concourse.bacc as bacc
import concourse.mybir as mybir
from concourse.tile import TileContext

F32 = mybir.dt.float32
BF16 = mybir.dt.bfloat16
AL = mybir.AluOpType
AF = mybir.ActivationFunctionType

B = 8
HD = 10
G = 6
HW = 192 * 192          # 36864 pixels
GP = HW // G            # 6144 pixels per group
CW = 1024               # chunk width (columns per group per chunk)
NCHUNK = GP // CW       # 6 chunks
EPS = 1e-5

NSTAT = 16
# S_BU broadcast: [h_att1 (rows 0:60); sum p_att1..4 (rows 64:124)]
# S_BL broadcast: [h_att2 (rows 0:60); sum p_att5..6 (rows 64:124)]
(S_BU, S_BL, S_CD, S_CU, S_CL,
 S_ZDA, S_ZDB, S_ZUA, S_ZUB, S_ZLB,
 S_GRM, S_GRH, S_GUM, S_GUH, S_GCM, S_GCRH) = range(NSTAT)

# conv block schedule: pairs of (cat-tile key, conv1 stat, conv2 stat).
# Each pair accumulates into one Z psum: first element -> z cols [0:60],
# second -> z cols [64:124]. Pairs 3/4 are single (odd number of upper z's).
PAIRS = [
    [("d0", S_CD, S_ZDA), ("d1", S_CD, S_ZDB)],
    [("c0", S_CU, S_ZUA), ("c4", S_CL, S_ZLB)],
    [("c1", S_CU, S_ZUA), ("c5", S_CL, S_ZLB)],
    [("c2", S_CU, S_ZUA)],
    [("c3", S_CU, S_ZUA)],
]


def _build_nc():
    nc = bacc.Bacc(trn_type="TRN2")

    catc = nc.declare_dram_parameter("catc", [6, 128, GP], BF16, isOutput=False)
    catd = nc.declare_dram_parameter("catd", [2, 128, GP], BF16, isOutput=False)
    xhbd = nc.declare_dram_parameter("xhb", [128, GP], BF16, isOutput=False)
    attd = nc.declare_dram_parameter("attb", [48, GP], BF16, isOutput=False)
    smtd = nc.declare_dram_parameter("smt", [128, NSTAT * 128], BF16,
                                     isOutput=False)
    outd = nc.declare_dram_parameter("out", [2, 60, GP], BF16, isOutput=True)

    with TileContext(nc) as tc:
        with (
            tc.tile_pool(name="const", bufs=1) as cpool,
            tc.tile_pool(name="cat", bufs=2) as catp,
            tc.tile_pool(name="hsb", bufs=3) as hsp,
            tc.tile_pool(name="chain", bufs=2) as chp,
            tc.tile_pool(name="msg", bufs=3) as msgp,
            tc.tile_pool(name="gate", bufs=2) as gatep,
            tc.tile_pool(name="outp", bufs=2) as outp,
            tc.tile_pool(name="xhp", bufs=3) as xhp,
        ):
            smt = cpool.tile([128, NSTAT * 128], BF16, name="smt")
            nc.sync.dma_start(out=smt[:, :], in_=smtd[:, :])
            atts = []
            for j in range(NCHUNK):
                a = cpool.tile([48, CW], BF16, name=f"att{j}")
                nc.sync.dma_start(out=a[:, :],
                                  in_=attd[:, j * CW:(j + 1) * CW])
                atts.append(a)

            def stat(i, K=128):
                return smt[0:K, i * 128:i * 128 + 128]

            def mm(ps, sidx, rhs_ap, start, stop, K=128):
                nc.tensor.matmul(ps, stat(sidx, K), rhs_ap,
                                 start=start, stop=stop)

            def loads(j, engs):
                t = {}
                names = [f"c{i}" for i in range(6)] + ["d0", "d1"]
                srcs = [catc[i] for i in range(6)] + [catd[0], catd[1]]
                for n, (name, src) in enumerate(zip(names, srcs)):
                    tl = catp.tile([128, CW], BF16, tag=name,
                                   name=f"{name}_{j}")
                    engs[n % len(engs)].dma_start(
                        out=tl[:, :], in_=src[:, j * CW:(j + 1) * CW])
                    t[name] = tl
                xh = xhp.tile([128, CW], BF16, tag="xh", name=f"xh_{j}")
                engs[0].dma_start(out=xh[:, :],
                                  in_=xhbd[:, j * CW:(j + 1) * CW])
                t["xh"] = xh
                return t

            # chunk-0/1 loads split across the idle-at-start queues
            tiles = {0: loads(0, [nc.scalar, nc.gpsimd]),
                     1: loads(1, [nc.sync])}

            # ---- attention broadcasts for all chunks (PSUM freed after) ----
            buts = [cpool.tile([128, CW], BF16, name=f"buts{j}")
                    for j in range(NCHUNK)]
            blts = [cpool.tile([128, CW], BF16, name=f"blts{j}")
                    for j in range(NCHUNK)]
            with tc.tile_pool(name="attpp", bufs=2, space="PSUM") as attpp:
                for j in range(NCHUNK):
                    for sidx, dst, nm in ((S_BU, buts[j], "pul"),
                                          (S_BL, blts[j], "pa")):
                        ps = attpp.tile([128, CW], F32, tag="aps",
                                        name=f"{nm}{j}")
                        for s in range(0, CW, 512):
                            mm(ps[0:128, s:s + 512], sidx,
                               atts[j][0:48, s:s + 512], True, True, K=48)
                        if nm == "pul":
                            nc.scalar.activation(dst[:, :], ps[:, :], AF.Copy)
                        else:
                            nc.vector.tensor_copy(dst[:, :], ps[:, :])

            def premults(j):
                # in-place on the cat tiles, Pool only
                t = tiles[j]
                for i in range(6):
                    src = t[f"c{i}"]
                    attv = (buts[j][64:124, :] if i < 4
                            else blts[j][64:124, :])
                    nc.gpsimd.tensor_tensor(src[64:124, :], src[64:124, :],
                                            attv, AL.mult)
                nc.gpsimd.tensor_tensor(t["d0"][0:60, :], t["d0"][0:60, :],
                                        buts[j][0:60, :], AL.mult)
                nc.gpsimd.tensor_tensor(t["d1"][0:60, :], t["d1"][0:60, :],
                                        blts[j][0:60, :], AL.mult)

            # ---- software-pipelined main loop ----
            # PSUM: hpp 2x[128,1024]=4 banks (H), zpp 1x[128,1024]=2 (Z),
            # gpp 1x[128,1024]=2 (GRU gates). Issue order per iteration:
            # loads/premults two chunks ahead, conv one ahead, then the
            # serial GRU tail of the current chunk, so tail ops never
            # head-of-line-block the next chunks' front work.
            with (
                tc.tile_pool(name="hpp", bufs=2, space="PSUM") as hpp,
                tc.tile_pool(name="zpp", bufs=1, space="PSUM") as zpp,
                tc.tile_pool(name="gpp", bufs=1, space="PSUM") as gpp,
            ):
                msgts = {}
                xhs = {}

                def conv(j):
                    t = tiles.pop(j)
                    xhs[j] = t["xh"]
                    schain = None
                    relu_k = 0
                    for zc, pair in enumerate(PAIRS):
                        zt = zpp.tile([128, CW], F32, tag="z",
                                      name=f"z{j}_{zc}")
                        for e, (key, w1, w2) in enumerate(pair):
                            hps = hpp.tile([128, CW], F32, tag="h",
                                           name=f"h{j}{key}")
                            for s in range(0, CW, 512):
                                mm(hps[0:128, s:s + 512], w1,
                                   t[key][0:128, s:s + 512], True, True)
                            hsb = hsp.tile([128, CW], BF16, tag="hs",
                                           name=f"hs{j}{key}")
                            if relu_k % 4 == 3:
                                nc.vector.tensor_scalar_max(
                                    hsb[:, :], hps[:, :], 0.0)
                            else:
                                nc.scalar.activation(hsb[:, :], hps[:, :],
                                                     AF.Relu)
                            relu_k += 1
                            for s in range(0, CW, 512):
                                mm(zt[0:128, s:s + 512], w2,
                                   hsb[0:128, s:s + 512],
                                   e == 0, e == len(pair) - 1)
                        if zc == 0:
                            so = chp.tile([128, CW], F32, tag="s",
                                          name=f"s{j}_{zc}")
                            nc.vector.tensor_scalar_max(
                                so[:, :], zt[:, :], 0.0)
                        elif zc < len(PAIRS) - 1:
                            so = chp.tile([128, CW], F32, tag="s",
                                          name=f"s{j}_{zc}")
                            nc.vector.scalar_tensor_tensor(
                                so[:, :], zt[:, :], 0.0, schain[:, :],
                                AL.max, AL.add)
                        else:
                            so = msgp.tile([128, CW], BF16, tag="msg",
                                           name=f"msg{j}")
                            nc.vector.scalar_tensor_tensor(
                                so[:, :], zt[:, :], 0.0, schain[:, :],
                                AL.max, AL.add)
                            msgts[j] = so
                        schain = so

                def tail(j):
                    msgt = msgts.pop(j)
                    xh = xhs.pop(j)
                    xsl = xh[:, :]
                    rt = gatep.tile([128, CW], BF16, tag="rt", name=f"rt{j}")
                    ut = gatep.tile([128, CW], BF16, tag="ut", name=f"ut{j}")
                    ct = gatep.tile([128, CW], BF16, tag="ct", name=f"ct{j}")
                    for sm, sh, dst, fn in ((S_GRM, S_GRH, rt, AF.Sigmoid),
                                            (S_GUM, S_GUH, ut, AF.Sigmoid)):
                        pg = gpp.tile([128, CW], F32, tag="g",
                                      name=f"g{j}{sm}")
                        for s in range(0, CW, 512):
                            mm(pg[0:128, s:s + 512], sm,
                               msgt[0:128, s:s + 512], True, False)
                            mm(pg[0:128, s:s + 512], sh,
                               xh[0:128, s:s + 512], False, True)
                        nc.scalar.activation(dst[:, :], pg[:, :], fn)
                    rht = gatep.tile([128, CW], BF16, tag="rh", name=f"rh{j}")
                    nc.gpsimd.tensor_tensor(rht[:, :], rt[:, :], xsl, AL.mult)
                    pg = gpp.tile([128, CW], F32, tag="g", name=f"gc{j}")
                    for s in range(0, CW, 512):
                        mm(pg[0:128, s:s + 512], S_GCM,
                           msgt[0:128, s:s + 512], True, False)
                        mm(pg[0:128, s:s + 512], S_GCRH,
                           rht[0:128, s:s + 512], False, True)
                    nc.scalar.activation(ct[:, :], pg[:, :], AF.Tanh)

                    # combine: out = h + u*(c - h)
                    dt = gatep.tile([128, CW], BF16, tag="dt", name=f"dt{j}")
                    nc.gpsimd.tensor_tensor(dt[:, :], ct[:, :], xsl,
                                            AL.subtract)
                    et = gatep.tile([128, CW], BF16, tag="et", name=f"et{j}")
                    nc.gpsimd.tensor_tensor(et[:, :], ut[:, :], dt[:, :],
                                            AL.mult)
                    ot = outp.tile([128, CW], BF16, tag="ot", name=f"ot{j}")
                    nc.vector.tensor_tensor(ot[:, :], xsl, et[:, :], AL.add)

                    nc.sync.dma_start(out=outd[0, :, j * CW:(j + 1) * CW],
                                      in_=ot[0:60, :])
                    nc.sync.dma_start(out=outd[1, :, j * CW:(j + 1) * CW],
                                      in_=ot[64:124, :])

                premults(0)
                premults(1)
                conv(0)
                for j in range(NCHUNK):
                    if j + 2 < NCHUNK:
                        tiles[j + 2] = loads(j + 2, [nc.sync])
                        premults(j + 2)
                    if j + 1 < NCHUNK:
                        conv(j + 1)
                    tail(j)

    nc.compile()
    return nc


def _fold(W, p):
    g, b, m, v = p[0], p[1], p[2], p[3]
    s = g / np.sqrt(v + EPS)
    return (s[:, None] * W).astype(np.float32), (b - m * s).astype(np.float32)


def _build_stats(dW1, dbn1, dW2, dbn2, uW1, ubn1, uW2, ubn2,
                 lW1, lbn1, lW2, lbn2, guWg, gubg, guWc, gubc,
                 glWg, glbg, glWc, glbc):
    dW1f, bd1 = _fold(dW1, dbn1)
    dW2f, bd2 = _fold(dW2, dbn2)
    uW1f, bu1 = _fold(uW1, ubn1)
    uW2f, bu2 = _fold(uW2, ubn2)
    lW1f, bl1 = _fold(lW1, lbn1)
    lW2f, bl2 = _fold(lW2, lbn2)

    S = np.zeros((NSTAT, 128, 128), np.float32)
    for g in range(G):
        r = 10 * g
        S[S_BU, g, r:r + 10] = 1.0          # h_att1 -> rows 0:60
        for k in (1, 2, 3, 4):              # sum p_att1..4 -> rows 64:124
            S[S_BU, 12 + 6 * (k - 1) + g, 64 + r:64 + r + 10] = 1.0
        S[S_BL, 6 + g, r:r + 10] = 1.0      # h_att2 -> rows 0:60
        for k in (5, 6):                    # sum p_att5..6 -> rows 64:124
            S[S_BL, 12 + 6 * (k - 1) + g, 64 + r:64 + r + 10] = 1.0

    def conv1(idx, Wf, bias):
        # cat rows [0:60]=first input (in-ch 0..9), [64:124]=second (10..19)
        for g in range(G):
            r = 10 * g
            S[idx, r:r + 10, r:r + 10] = Wf[0:10, 0:10].T
            S[idx, r:r + 10, 64 + r:64 + r + 10] = Wf[10:20, 0:10].T
            S[idx, 64 + r:64 + r + 10, r:r + 10] = Wf[0:10, 10:20].T
            S[idx, 64 + r:64 + r + 10, 64 + r:64 + r + 10] = Wf[10:20, 10:20].T
            S[idx, 60, r:r + 10] = bias[0:10]
            S[idx, 60, 64 + r:64 + r + 10] = bias[10:20]
        S[idx, 60, 60] = 1.0    # H ones-row for conv2 bias injection

    conv1(S_CD, dW1f, bd1)
    conv1(S_CU, uW1f, bu1)
    conv1(S_CL, lW1f, bl1)

    def conv2(idx, Wf, bias, off, ones):
        for g in range(G):
            r = 10 * g
            S[idx, r:r + 10, off + r:off + r + 10] = Wf[:, 0:10].T
            S[idx, 64 + r:64 + r + 10, off + r:off + r + 10] = Wf[:, 10:20].T
            S[idx, 60, off + r:off + r + 10] = bias
        if ones:
            S[idx, 60, 60] = 1.0    # msg ones-row for GRU bias injection

    conv2(S_ZDA, dW2f, bd2, 0, True)
    conv2(S_ZDB, dW2f, bd2, 64, False)
    conv2(S_ZUA, uW2f, bu2, 0, False)
    conv2(S_ZUB, uW2f, bu2, 64, False)
    conv2(S_ZLB, lW2f, bl2, 64, False)

    def gru(idx, Wu, Wl, rows, incol, bu_, bl_):
        for g in range(G):
            r = 10 * g
            S[idx, r:r + 10, r:r + 10] = Wu[rows, incol:incol + 10].T
            S[idx, 64 + r:64 + r + 10, 64 + r:64 + r + 10] = \
                Wl[rows, incol:incol + 10].T
            if bu_ is not None:
                S[idx, 60, r:r + 10] = bu_
                S[idx, 60, 64 + r:64 + r + 10] = bl_

    gru(S_GRM, guWg, glWg, slice(0, 10), 0, gubg[0:10], glbg[0:10])
    gru(S_GRH, guWg, glWg, slice(0, 10), 10, None, None)
    gru(S_GUM, guWg, glWg, slice(10, 20), 0, gubg[10:20], glbg[10:20])
    gru(S_GUH, guWg, glWg, slice(10, 20), 10, None, None)
    gru(S_GCM, guWc, glWc, slice(0, 10), 0, gubc, glbc)
    gru(S_GCRH, guWc, glWc, slice(0, 10), 10, None, None)
    return S


BF_NP = mybir.dt.np(mybir.dt.bfloat16)


def _planar(a):
    # [HD, H, W] -> [60, GP]: row 10*g + c
    a = np.asarray(a, np.float32).reshape(HD, G, GP)
    return np.moveaxis(a, 1, 0).reshape(60, GP)


def _unplanar(a):
    # [2, 60, GP] -> [2, HD, 192, 192]
    a = a.reshape(2, G, HD, GP)
    return np.moveaxis(a, 1, 2).reshape(2, HD, 192, 192)


def make_in_maps(xf, xh, xp, h_att, p_att, S):
    smt = np.ascontiguousarray(
        S.transpose(1, 0, 2).reshape(128, NSTAT * 128)).astype(BF_NP)
    in_maps = []
    for b in range(B):
        xfp = _planar(xf[b])
        xhu = _planar(xh[0, b])
        xhl = _planar(xh[1, b])
        catc = np.zeros((6, 128, GP), np.float32)
        for i in range(6):
            catc[i, 0:60] = xhu if i < 4 else xhl
            catc[i, 60] = 1.0
            catc[i, 64:124] = _planar(xp[i, b])
        catd = np.zeros((2, 128, GP), np.float32)
        for i, xh_half in enumerate((xhu, xhl)):
            catd[i, 0:60] = xfp
            catd[i, 60] = 1.0
            catd[i, 64:124] = xh_half
        xhb = np.zeros((128, GP), np.float32)
        xhb[0:60] = xhu
        xhb[64:124] = xhl
        attb = np.zeros((48, GP), np.float32)
        attb[0:6] = h_att[1, b, 0].reshape(G, GP)
        attb[6:12] = h_att[2, b, 0].reshape(G, GP)
        for k in range(1, 7):
            attb[12 + 6 * (k - 1):12 + 6 * k] = p_att[k, b, 0].reshape(G, GP)
        in_maps.append(dict(
            catc=np.ascontiguousarray(catc).astype(BF_NP),
            catd=np.ascontiguousarray(catd).astype(BF_NP),
            xhb=np.ascontiguousarray(xhb).astype(BF_NP),
            attb=np.ascontiguousarray(attb).astype(BF_NP),
            smt=smt,
        ))
    return in_maps


_NC_CACHE = None


def _get_nc():
    global _NC_CACHE
    if _NC_CACHE is None:
        _NC_CACHE = _build_nc()
    return _NC_CACHE


def _prep(xf, xh, xp, h_att, p_att,
          dW1, dbn1, dW2, dbn2, uW1, ubn1, uW2, ubn2,
          lW1, lbn1, lW2, lbn2, guWg, gubg, guWc, gubc,
          glWg, glbg, glWc, glbc):
    args = [np.asarray(a, dtype=np.float32) for a in
            (dW1, dbn1, dW2, dbn2, uW1, ubn1, uW2, ubn2,
             lW1, lbn1, lW2, lbn2, guWg, gubg, guWc, gubc,
             glWg, glbg, glWc, glbc)]
    S = _build_stats(*args)
    return make_in_maps(np.asarray(xf, np.float32), np.asarray(xh, np.float32),
                        np.asarray(xp, np.float32),
                        np.asarray(h_att, np.float32),
                        np.asarray(p_att, np.float32), S)


def kernel(xf, xh, xp, h_att, p_att,
           dW1, dbn1, dW2, dbn2,
           uW1, ubn1, uW2, ubn2,
           lW1, lbn1, lW2, lbn2,
           guWg, gubg, guWc, gubc,
           glWg, glbg, glWc, glbc,
           _trace=False):
    from concourse.bass_utils import run_bass_kernel_spmd

    in_maps = _prep(xf, xh, xp, h_att, p_att,
                    dW1, dbn1, dW2, dbn2, uW1, ubn1, uW2, ubn2,
                    lW1, lbn1, lW2, lbn2, guWg, gubg, guWc, gubc,
                    glWg, glbg, glWc, glbc)
    nc = _get_nc()
    res = run_bass_kernel_spmd(nc, in_maps, core_ids=list(range(B)),
                               trace=_trace)
    out = np.empty((2, B, HD, 192, 192), np.float32)
    for b in range(B):
        out[:, b] = _unplanar(np.asarray(res.results[b]["out"], np.float32))
    if _trace:
        return out, res
    return out


# revision 15
# speedup vs baseline: 1.0267x; 1.0267x over previous
"""Trainium2 Bass kernel for nn_Half_Graph (GNN message passing block).

Data-parallel over batch: core b processes image b (B=8 across 8 cores).

Planar layout: SBUF partition 10*g + c <-> (channel c, pixel group g) with
G=6 groups of 6144 pixels; a 10-channel tensor occupies 60 partitions.
128-partition tiles hold two such 60-row halves at [0:60] and [64:124].

All convs are 1x1 -> matmuls with block-diagonal stationaries. Host builds
bf16 concat tensors [first; second] per conv block so each conv1 is a
SINGLE matmul pass (K=128). Partition row 60 of every concat tensor is a
constant 1.0, and stationary row 60 carries the folded BN / GRU biases, so
biases ride along in the matmul for free. The attention premultiply reads
a PE-broadcast copy of the attention maps (computed once for all chunks up
front). The inter-block message sum is a relu+add chain on Pool/DVE
reading conv2 PSUM tiles directly. GRU output uses out = h + u*(c - h).

Host side pre-transposes/concats/casts (cheap, not part of measured
device time) so every DMA is a plain 2D bf16 slice.
"""

import sys

for _p in ("/opt/trn_rl_repo", "/root/.axon_site/_ro/trn_rl_repo"):
    if _p not in sys.path:
        sys.path.insert(0, _p)

import numpy as np

import concourse.bass as bass
import concourse.bacc as bacc
import concourse.mybir as mybir
from concourse.tile import TileContext

F32 = mybir.dt.float32
BF16 = mybir.dt.bfloat16
AL = mybir.AluOpType
AF = mybir.ActivationFunctionType

B = 8
HD = 10
G = 6
HW = 192 * 192          # 36864 pixels
GP = HW // G            # 6144 pixels per group
CW = 1024               # chunk width (columns per group per chunk)
NCHUNK = GP // CW       # 6 chunks
EPS = 1e-5

NSTAT = 16
# S_BU broadcast: [h_att1 (rows 0:60); sum p_att1..4 (rows 64:124)]
# S_BL broadcast: [h_att2 (rows 0:60); sum p_att5..6 (rows 64:124)]
(S_BU, S_BL, S_CD, S_CU, S_CL,
 S_ZDA, S_ZDB, S_ZUA, S_ZUB, S_ZLB,
 S_GRM, S_GRH, S_GUM, S_GUH, S_GCM, S_GCRH) = range(NSTAT)

# conv block schedule: pairs of (cat-tile key, conv1 stat, conv2 stat).
# Each pair accumulates into one Z psum: first element -> z cols [0:60],
# second -> z cols [64:124]. Pairs 3/4 are single (odd number of upper z's).
PAIRS = [
    [("d0", S_CD, S_ZDA), ("d1", S_CD, S_ZDB)],
    [("c0", S_CU, S_ZUA), ("c4", S_CL, S_ZLB)],
    [("c1", S_CU, S_ZUA), ("c5", S_CL, S_ZLB)],
    [("c2", S_CU, S_ZUA)],
    [("c3", S_CU, S_ZUA)],
]


def _build_nc():
    nc = bacc.Bacc(trn_type="TRN2")

    catc = nc.declare_dram_parameter("catc", [6, 128, GP], BF16, isOutput=False)
    catd = nc.declare_dram_parameter("catd", [2, 128, GP], BF16, isOutput=False)
    xhbd = nc.declare_dram_parameter("xhb", [128, GP], BF16, isOutput=False)
    attd = nc.declare_dram_parameter("attb", [48, GP], BF16, isOutput=False)
    smtd = nc.declare_dram_parameter("smt", [128, NSTAT * 128], BF16,
                                     isOutput=False)
    outd = nc.declare_dram_parameter("out", [2, 60, GP], BF16, isOutput=True)

    with TileContext(nc) as tc:
        with (
            tc.tile_pool(name="const", bufs=1) as cpool,
            tc.tile_pool(name="cat", bufs=2) as catp,
            tc.tile_pool(name="hsb", bufs=3) as hsp,
            tc.tile_pool(name="chain", bufs=2) as chp,
            tc.tile_pool(name="msg", bufs=3) as msgp,
            tc.tile_pool(name="gate", bufs=2) as gatep,
            tc.tile_pool(name="outp", bufs=2) as outp,
            tc.tile_pool(name="xhp", bufs=3) as xhp,
        ):
            smt = cpool.tile([128, NSTAT * 128], BF16, name="smt")
            nc.sync.dma_start(out=smt[:, :], in_=smtd[:, :])
            atts = []
            for j in range(NCHUNK):
                a = cpool.tile([48, CW], BF16, name=f"att{j}")
                nc.sync.dma_start(out=a[:, :],
                                  in_=attd[:, j * CW:(j + 1) * CW])
                atts.append(a)

            def stat(i, K=128):
                return smt[0:K, i * 128:i * 128 + 128]

            def mm(ps, sidx, rhs_ap, start, stop, K=128):
                nc.tensor.matmul(ps, stat(sidx, K), rhs_ap,
                                 start=start, stop=stop)

            def loads(j, engs):
                t = {}
                names = [f"c{i}" for i in range(6)] + ["d0", "d1"]
                srcs = [catc[i] for i in range(6)] + [catd[0], catd[1]]
                for n, (name, src) in enumerate(zip(names, srcs)):
                    tl = catp.tile([128, CW], BF16, tag=name,
                                   name=f"{name}_{j}")
                    engs[n % len(engs)].dma_start(
                        out=tl[:, :], in_=src[:, j * CW:(j + 1) * CW])
                    t[name] = tl
                xh = xhp.tile([128, CW], BF16, tag="xh", name=f"xh_{j}")
                engs[0].dma_start(out=xh[:, :],
                                  in_=xhbd[:, j * CW:(j + 1) * CW])
                t["xh"] = xh
                return t

            # chunk-0/1 loads split across the idle-at-start queues
            tiles = {0: loads(0, [nc.scalar, nc.gpsimd]),
                     1: loads(1, [nc.sync])}

            # ---- attention broadcasts for all chunks (PSUM freed after) ----
            buts = [cpool.tile([128, CW], BF16, name=f"buts{j}")
                    for j in range(NCHUNK)]
            blts = [cpool.tile([128, CW], BF16, name=f"blts{j}")
                    for j in range(NCHUNK)]
            with tc.tile_pool(name="attpp", bufs=2, space="PSUM") as attpp:
                for j in range(NCHUNK):
                    for sidx, dst, nm in ((S_BU, buts[j], "pul"),
                                          (S_BL, blts[j], "pa")):
                        ps = attpp.tile([128, CW], F32, tag="aps",
                                        name=f"{nm}{j}")
                        for s in range(0, CW, 512):
                            mm(ps[0:128, s:s + 512], sidx,
                               atts[j][0:48, s:s + 512], True, True, K=48)
                        if nm == "pul":
                            nc.scalar.activation(dst[:, :], ps[:, :], AF.Copy)
                        else:
                            nc.vector.tensor_copy(dst[:, :], ps[:, :])

            def premults(j):
                # in-place on the cat tiles, Pool only
                t = tiles[j]
                for i in range(6):
                    src = t[f"c{i}"]
                    attv = (buts[j][64:124, :] if i < 4
                            else blts[j][64:124, :])
                    nc.gpsimd.tensor_tensor(src[64:124, :], src[64:124, :],
                                            attv, AL.mult)
                nc.gpsimd.tensor_tensor(t["d0"][0:60, :], t["d0"][0:60, :],
                                        buts[j][0:60, :], AL.mult)
                nc.gpsimd.tensor_tensor(t["d1"][0:60, :], t["d1"][0:60, :],
                                        blts[j][0:60, :], AL.mult)

            # ---- software-pipelined main loop ----
            # PSUM: hpp 2x[128,1024]=4 banks (H), zpp 1x[128,1024]=2 (Z),
            # gpp 1x[128,1024]=2 (GRU gates). Issue order per iteration:
            # loads/premults two chunks ahead, conv one ahead, then the
            # serial GRU tail of the current chunk, so tail ops never
            # head-of-line-block the next chunks' front work.
            with (
                tc.tile_pool(name="hpp", bufs=2, space="PSUM") as hpp,
                tc.tile_pool(name="zpp", bufs=1, space="PSUM") as zpp,
                tc.tile_pool(name="gpp", bufs=1, space="PSUM") as gpp,
            ):
                msgts = {}
                xhs = {}

                def conv(j):
                    t = tiles.pop(j)
                    xhs[j] = t["xh"]
                    # two independent relu+add chains, one per 512-col
                    # sub-half, to halve the serial dependency depth
                    schain = [None, None]
                    msgt = msgp.tile([128, CW], BF16, tag="msg",
                                     name=f"msg{j}")
                    relu_k = 0
                    for zc, pair in enumerate(PAIRS):
                        zts = [zpp.tile([128, 512], F32, tag=f"zs{si}",
                                        name=f"z{j}_{zc}_{si}")
                               for si in range(2)]
                        for e, (key, w1, w2) in enumerate(pair):
                            hps = hpp.tile([128, CW], F32, tag="h",
                                           name=f"h{j}{key}")
                            for s in range(0, CW, 512):
                                mm(hps[0:128, s:s + 512], w1,
                                   t[key][0:128, s:s + 512], True, True)
                            hsb = hsp.tile([128, CW], BF16, tag="hs",
                                           name=f"hs{j}{key}")
                            if relu_k % 4 == 3:
                                nc.vector.tensor_scalar_max(
                                    hsb[:, :], hps[:, :], 0.0)
                            else:
                                nc.scalar.activation(hsb[:, :], hps[:, :],
                                                     AF.Relu)
                            relu_k += 1
                            for si in range(2):
                                s = si * 512
                                mm(zts[si][0:128, 0:512], w2,
                                   hsb[0:128, s:s + 512],
                                   e == 0, e == len(pair) - 1)
                        for si in range(2):
                            s = si * 512
                            if zc == 0:
                                so = chp.tile([128, 512], F32, tag=f"s{si}",
                                              name=f"s{j}_{zc}_{si}")
                                nc.vector.tensor_scalar_max(
                                    so[:, :], zts[si][:, :], 0.0)
                            elif zc < len(PAIRS) - 1:
                                so = chp.tile([128, 512], F32, tag=f"s{si}",
                                              name=f"s{j}_{zc}_{si}")
                                nc.vector.scalar_tensor_tensor(
                                    so[:, :], zts[si][:, :], 0.0,
                                    schain[si][:, :], AL.max, AL.add)
                            else:
                                so = None
                                nc.vector.scalar_tensor_tensor(
                                    msgt[:, s:s + 512], zts[si][:, :], 0.0,
                                    schain[si][:, :], AL.max, AL.add)
                            schain[si] = so
                    msgts[j] = msgt

                def tail(j):
                    msgt = msgts.pop(j)
                    xh = xhs.pop(j)
                    xsl = xh[:, :]
                    rt = gatep.tile([128, CW], BF16, tag="rt", name=f"rt{j}")
                    ut = gatep.tile([128, CW], BF16, tag="ut", name=f"ut{j}")
                    ct = gatep.tile([128, CW], BF16, tag="ct", name=f"ct{j}")
                    for sm, sh, dst, fn in ((S_GRM, S_GRH, rt, AF.Sigmoid),
                                            (S_GUM, S_GUH, ut, AF.Sigmoid)):
                        pg = gpp.tile([128, CW], F32, tag="g",
                                      name=f"g{j}{sm}")
                        for s in range(0, CW, 512):
                            mm(pg[0:128, s:s + 512], sm,
                               msgt[0:128, s:s + 512], True, False)
                            mm(pg[0:128, s:s + 512], sh,
                               xh[0:128, s:s + 512], False, True)
                        nc.scalar.activation(dst[:, :], pg[:, :], fn)
                    rht = gatep.tile([128, CW], BF16, tag="rh", name=f"rh{j}")
                    nc.gpsimd.tensor_tensor(rht[:, :], rt[:, :], xsl, AL.mult)
                    pg = gpp.tile([128, CW], F32, tag="g", name=f"gc{j}")
                    for s in range(0, CW, 512):
                        mm(pg[0:128, s:s + 512], S_GCM,
                           msgt[0:128, s:s + 512], True, False)
                        mm(pg[0:128, s:s + 512], S_GCRH,
                           rht[0:128, s:s + 512], False, True)
                    nc.scalar.activation(ct[:, :], pg[:, :], AF.Tanh)

                    # combine: out = h + u*(c - h)
                    dt = gatep.tile([128, CW], BF16, tag="dt", name=f"dt{j}")
                    nc.gpsimd.tensor_tensor(dt[:, :], ct[:, :], xsl,
                                            AL.subtract)
                    et = gatep.tile([128, CW], BF16, tag="et", name=f"et{j}")
                    nc.gpsimd.tensor_tensor(et[:, :], ut[:, :], dt[:, :],
                                            AL.mult)
                    ot = outp.tile([128, CW], BF16, tag="ot", name=f"ot{j}")
                    nc.vector.tensor_tensor(ot[:, :], xsl, et[:, :], AL.add)

                    nc.sync.dma_start(out=outd[0, :, j * CW:(j + 1) * CW],
                                      in_=ot[0:60, :])
                    nc.sync.dma_start(out=outd[1, :, j * CW:(j + 1) * CW],
                                      in_=ot[64:124, :])

                premults(0)
                premults(1)
                conv(0)
                for j in range(NCHUNK):
                    if j + 2 < NCHUNK:
                        tiles[j + 2] = loads(j + 2, [nc.sync])
                        premults(j + 2)
                    if j + 1 < NCHUNK:
                        conv(j + 1)
                    tail(j)

    nc.compile()
    return nc


def _fold(W, p):
    g, b, m, v = p[0], p[1], p[2], p[3]
    s = g / np.sqrt(v + EPS)
    return (s[:, None] * W).astype(np.float32), (b - m * s).astype(np.float32)


def _build_stats(dW1, dbn1, dW2, dbn2, uW1, ubn1, uW2, ubn2,
                 lW1, lbn1, lW2, lbn2, guWg, gubg, guWc, gubc,
                 glWg, glbg, glWc, glbc):
    dW1f, bd1 = _fold(dW1, dbn1)
    dW2f, bd2 = _fold(dW2, dbn2)
    uW1f, bu1 = _fold(uW1, ubn1)
    uW2f, bu2 = _fold(uW2, ubn2)
    lW1f, bl1 = _fold(lW1, lbn1)
    lW2f, bl2 = _fold(lW2, lbn2)

    S = np.zeros((NSTAT, 128, 128), np.float32)
    for g in range(G):
        r = 10 * g
        S[S_BU, g, r:r + 10] = 1.0          # h_att1 -> rows 0:60
        for k in (1, 2, 3, 4):              # sum p_att1..4 -> rows 64:124
            S[S_BU, 12 + 6 * (k - 1) + g, 64 + r:64 + r + 10] = 1.0
        S[S_BL, 6 + g, r:r + 10] = 1.0      # h_att2 -> rows 0:60
        for k in (5, 6):                    # sum p_att5..6 -> rows 64:124
            S[S_BL, 12 + 6 * (k - 1) + g, 64 + r:64 + r + 10] = 1.0

    def conv1(idx, Wf, bias):
        # cat rows [0:60]=first input (in-ch 0..9), [64:124]=second (10..19)
        for g in range(G):
            r = 10 * g
            S[idx, r:r + 10, r:r + 10] = Wf[0:10, 0:10].T
            S[idx, r:r + 10, 64 + r:64 + r + 10] = Wf[10:20, 0:10].T
            S[idx, 64 + r:64 + r + 10, r:r + 10] = Wf[0:10, 10:20].T
            S[idx, 64 + r:64 + r + 10, 64 + r:64 + r + 10] = Wf[10:20, 10:20].T
            S[idx, 60, r:r + 10] = bias[0:10]
            S[idx, 60, 64 + r:64 + r + 10] = bias[10:20]
        S[idx, 60, 60] = 1.0    # H ones-row for conv2 bias injection

    conv1(S_CD, dW1f, bd1)
    conv1(S_CU, uW1f, bu1)
    conv1(S_CL, lW1f, bl1)

    def conv2(idx, Wf, bias, off, ones):
        for g in range(G):
            r = 10 * g
            S[idx, r:r + 10, off + r:off + r + 10] = Wf[:, 0:10].T
            S[idx, 64 + r:64 + r + 10, off + r:off + r + 10] = Wf[:, 10:20].T
            S[idx, 60, off + r:off + r + 10] = bias
        if ones:
            S[idx, 60, 60] = 1.0    # msg ones-row for GRU bias injection

    conv2(S_ZDA, dW2f, bd2, 0, True)
    conv2(S_ZDB, dW2f, bd2, 64, False)
    conv2(S_ZUA, uW2f, bu2, 0, False)
    conv2(S_ZUB, uW2f, bu2, 64, False)
    conv2(S_ZLB, lW2f, bl2, 64, False)

    def gru(idx, Wu, Wl, rows, incol, bu_, bl_):
        for g in range(G):
            r = 10 * g
            S[idx, r:r + 10, r:r + 10] = Wu[rows, incol:incol + 10].T
            S[idx, 64 + r:64 + r + 10, 64 + r:64 + r + 10] = \
                Wl[rows, incol:incol + 10].T
            if bu_ is not None:
                S[idx, 60, r:r + 10] = bu_
                S[idx, 60, 64 + r:64 + r + 10] = bl_

    gru(S_GRM, guWg, glWg, slice(0, 10), 0, gubg[0:10], glbg[0:10])
    gru(S_GRH, guWg, glWg, slice(0, 10), 10, None, None)
    gru(S_GUM, guWg, glWg, slice(10, 20), 0, gubg[10:20], glbg[10:20])
    gru(S_GUH, guWg, glWg, slice(10, 20), 10, None, None)
    gru(S_GCM, guWc, glWc, slice(0, 10), 0, gubc, glbc)
    gru(S_GCRH, guWc, glWc, slice(0, 10), 10, None, None)
    return S


BF_NP = mybir.dt.np(mybir.dt.bfloat16)


def _planar(a):
    # [HD, H, W] -> [60, GP]: row 10*g + c
    a = np.asarray(a, np.float32).reshape(HD, G, GP)
    return np.moveaxis(a, 1, 0).reshape(60, GP)


def _unplanar(a):
    # [2, 60, GP] -> [2, HD, 192, 192]
    a = a.reshape(2, G, HD, GP)
    return np.moveaxis(a, 1, 2).reshape(2, HD, 192, 192)


def make_in_maps(xf, xh, xp, h_att, p_att, S):
    smt = np.ascontiguousarray(
        S.transpose(1, 0, 2).reshape(128, NSTAT * 128)).astype(BF_NP)
    in_maps = []
    for b in range(B):
        xfp = _planar(xf[b])
        xhu = _planar(xh[0, b])
        xhl = _planar(xh[1, b])
        catc = np.zeros((6, 128, GP), np.float32)
        for i in range(6):
            catc[i, 0:60] = xhu if i < 4 else xhl
            catc[i, 60] = 1.0
            catc[i, 64:124] = _planar(xp[i, b])
        catd = np.zeros((2, 128, GP), np.float32)
        for i, xh_half in enumerate((xhu, xhl)):
            catd[i, 0:60] = xfp
            catd[i, 60] = 1.0
            catd[i, 64:124] = xh_half
        xhb = np.zeros((128, GP), np.float32)
        xhb[0:60] = xhu
        xhb[64:124] = xhl
        attb = np.zeros((48, GP), np.float32)
        attb[0:6] = h_att[1, b, 0].reshape(G, GP)
        attb[6:12] = h_att[2, b, 0].reshape(G, GP)
        for k in range(1, 7):
            attb[12 + 6 * (k - 1):12 + 6 * k] = p_att[k, b, 0].reshape(G, GP)
        in_maps.append(dict(
            catc=np.ascontiguousarray(catc).astype(BF_NP),
            catd=np.ascontiguousarray(catd).astype(BF_NP),
            xhb=np.ascontiguousarray(xhb).astype(BF_NP),
            attb=np.ascontiguousarray(attb).astype(BF_NP),
            smt=smt,
        ))
    return in_maps


_NC_CACHE = None


def _get_nc():
    global _NC_CACHE
    if _NC_CACHE is None:
        _NC_CACHE = _build_nc()
    return _NC_CACHE


def _prep(xf, xh, xp, h_att, p_att,
          dW1, dbn1, dW2, dbn2, uW1, ubn1, uW2, ubn2,
          lW1, lbn1, lW2, lbn2, guWg, gubg, guWc, gubc,
          glWg, glbg, glWc, glbc):
    args = [np.asarray(a, dtype=np.float32) for a in
            (dW1, dbn1, dW2, dbn2, uW1, ubn1, uW2, ubn2,
             lW1, lbn1, lW2, lbn2, guWg, gubg, guWc, gubc,
             glWg, glbg, glWc, glbc)]
    S = _build_stats(*args)
    return make_in_maps(np.asarray(xf, np.float32), np.asarray(xh, np.float32),
                        np.asarray(xp, np.float32),
                        np.asarray(h_att, np.float32),
                        np.asarray(p_att, np.float32), S)


def kernel(xf, xh, xp, h_att, p_att,
           dW1, dbn1, dW2, dbn2,
           uW1, ubn1, uW2, ubn2,
           lW1, lbn1, lW2, lbn2,
           guWg, gubg, guWc, gubc,
           glWg, glbg, glWc, glbc,
           _trace=False):
    from concourse.bass_utils import run_bass_kernel_spmd

    in_maps = _prep(xf, xh, xp, h_att, p_att,
                    dW1, dbn1, dW2, dbn2, uW1, ubn1, uW2, ubn2,
                    lW1, lbn1, lW2, lbn2, guWg, gubg, guWc, gubc,
                    glWg, glbg, glWc, glbc)
    nc = _get_nc()
    res = run_bass_kernel_spmd(nc, in_maps, core_ids=list(range(B)),
                               trace=_trace)
    out = np.empty((2, B, HD, 192, 192), np.float32)
    for b in range(B):
        out[:, b] = _unplanar(np.asarray(res.results[b]["out"], np.float32))
    if _trace:
        return out, res
    return out
